# revision 14
# baseline (speedup 1.0000x reference)
"""Trainium2 Bass kernel for nn_LowFreqCrossAttn (dense transformer cross-attention).

Data-parallel over batch: 16 batches -> 8 NeuronCores, 2 batches/core.
Weights / attention-bias tables replicated.

v2 schedule: projections are interleaved into the attention stream (no
serial phase A), the Scalar engine runs exp only (all PSUM evacuations on
DVE/GpSimd), and the 16-row m-tail (784 = 6*128 + 16) is batched across
heads: per batch, two 4-head tail groups each get ONE exp + ONE bias-mult
over a shared [128, N] tile (head j of a group at partitions 32j..32j+16,
satisfying the PE tile_position 32-alignment rule for 16-row outputs).

Per-core dataflow (matmuls fp16 x fp16 -> f32 PSUM):
  proj) q = (s*Wq) @ ll, k = (0.5*Wk) @ ha  (head rows duplicated to K=128
     to keep the PE HAM clock warm; gpsimd evac with per-partition bias)
     vT = ha^T @ WvT (token-major 80-col head blocks, ones col @64 via
     memset; v-bias folded into proj bias on host; tail rows replicated
     at partitions 32/64/96 for the tail-group AV matmuls)
  attn, per (head, batch): logitsT = k_h^T q_h -> exp (ACT) -> *exp_bias
     (DVE) -> out_unT accumulated in PSUM over tail + 6 m-tiles
  softmax tails (batched, DRAM-bounce broadcast) + y = WpT^T @ onorm + b
     as in v1, but emitted so b1's last heads are never queued behind
     b0's tail/proj work.
"""

import numpy as np

B = 16
C = 384
RES = 28
N = 784
NH = 8
HD = 48
NP = 392            # n-chunk (half of N; fits one PSUM bank in f32)
NCORES = 8
BPC = 2             # batches per core
SCALE = HD ** -0.5
NMT = 6             # full 128-row m-tiles; rows 768:784 are the tail

TRACE = False       # set True to capture an NTFF trace on core 0
LAST_RESULTS = {}   # exec_time_ns etc. from the last run (when TRACE)

_CACHE = {}


def _build_nc():
    import concourse.bacc as bacc
    import concourse.mybir as mybir
    import concourse.tile as tile

    f16 = mybir.dt.float16
    f32 = mybir.dt.float32
    AF = mybir.ActivationFunctionType
    MUL = mybir.AluOpType.mult
    ADD = mybir.AluOpType.add

    nc = bacc.Bacc("TRN2", target_bir_lowering=False, debug=False)

    ll_d = nc.declare_dram_parameter("ll", [BPC, C, N], f16, isOutput=False)
    ha_d = nc.declare_dram_parameter("ha", [BPC, C, N], f16, isOutput=False)
    qwT_d = nc.declare_dram_parameter("qwT", [3, 128, 512], f16, isOutput=False)
    kwT_d = nc.declare_dram_parameter("kwT", [3, 128, 512], f16, isOutput=False)
    vwT_d = nc.declare_dram_parameter("vwT", [3, 128, 384], f16, isOutput=False)
    pwT_d = nc.declare_dram_parameter("pwT", [4, 128, 384], f16, isOutput=False)
    qb_d = nc.declare_dram_parameter("qb", [128, 4], f32, isOutput=False)
    kb_d = nc.declare_dram_parameter("kb", [128, 4], f32, isOutput=False)
    pb_d = nc.declare_dram_parameter("pb", [128, 3], f32, isOutput=False)
    expb_d = nc.declare_dram_parameter("expb", [NH, NMT, 128, N], f16, isOutput=False)
    expbt_d = nc.declare_dram_parameter("expbt", [3, 128, N], f16, isOutput=False)
    out_d = nc.declare_dram_parameter("out", [BPC, C, N], f32, isOutput=True)

    with tile.TileContext(nc) as tc:
        with (
            tc.tile_pool(name="const", bufs=1) as cp,
            tc.tile_pool(name="persist", bufs=1) as pp,
            tc.tile_pool(name="dram", bufs=1, space="DRAM") as dp,
            tc.tile_pool(name="ebp", bufs=3) as ebp,
            tc.tile_pool(name="etp", bufs=2) as etp,
            tc.tile_pool(name="psqk", bufs=2, space="PSUM") as psqk,
            tc.tile_pool(name="psav", bufs=2, space="PSUM") as psav,
            tc.tile_pool(name="yp", bufs=2) as ypool,
        ):
            # ---- ACT exp-table preload (hide the ~2.7us table load) ----
            sc1 = cp.tile([1, 8], f16, tag="sc1", name="sc1")
            sc2 = cp.tile([1, 8], f16, tag="sc2", name="sc2")
            nc.gpsimd.memset(sc1[:], 0.0)
            nc.scalar.activation(sc2[:], sc1[:], AF.Exp)

            # ---- constant loads ----
            qwT_sb = [cp.tile([128, 512], f16, tag=f"qwT{t}", name=f"qwT{t}") for t in range(3)]
            kwT_sb = [cp.tile([128, 512], f16, tag=f"kwT{t}", name=f"kwT{t}") for t in range(3)]
            vwT_sb = [cp.tile([128, 384], f16, tag=f"vwT{t}", name=f"vwT{t}") for t in range(3)]
            pwT_sb = [cp.tile([128, 384], f16, tag=f"pwT{p}", name=f"pwT{p}") for p in range(4)]
            qb_sb = cp.tile([128, 4], f32, tag="qb", name="qb")
            kb_sb = cp.tile([128, 4], f32, tag="kb", name="kb")
            pb_sb = cp.tile([128, 3], f32, tag="pb", name="pb")
            for t in range(3):
                nc.sync.dma_start(qwT_sb[t][:], qwT_d[t])
                nc.sync.dma_start(kwT_sb[t][:], kwT_d[t])
                nc.sync.dma_start(vwT_sb[t][:], vwT_d[t])
            for p in range(4):
                nc.sync.dma_start(pwT_sb[p][:], pwT_d[p])
            nc.sync.dma_start(qb_sb[:], qb_d[:])
            nc.sync.dma_start(kb_sb[:], kb_d[:])
            nc.sync.dma_start(pb_sb[:], pb_d[:])

            # ---- activations in (b0 first, 4-way split for DMA parallelism) ----
            ll_sb = [pp.tile([128, 3, N], f16, tag=f"ll{b}", name=f"ll{b}") for b in range(BPC)]
            ha_sb = [pp.tile([128, 3, N], f16, tag=f"ha{b}", name=f"ha{b}") for b in range(BPC)]

            def load_act(b):
                NQ = 196
                for src_d, dst in ((ll_d, ll_sb[b]), (ha_d, ha_sb[b])):
                    src = src_d[b].rearrange("(t p) n -> p t n", p=128)
                    for qd in range(4):
                        nc.sync.dma_start(dst[:, :, NQ * qd:NQ * (qd + 1)],
                                          src[:, :, NQ * qd:NQ * (qd + 1)])

            load_act(0)

            # exp-bias tiles: full m-tiles per head + shared tail groups
            eb_tiles = {}

            def load_eb(h):
                eb_sb = ebp.tile([128, NMT, N], f16, tag="eb", name=f"eb{h}")
                for t in range(NMT):
                    nc.sync.dma_start(eb_sb[:, t, :], expb_d[h, t])
                eb_tiles[h] = eb_sb

            load_eb(0)
            load_eb(1)
            # tail groups: heads at partition offsets {0, 32, 64} (base
            # partition 96 is not addressable by matmul tile_position)
            TGROUPS = [(0, 1, 2), (3, 4, 5), (6, 7)]
            TG = {h: (g, j) for g, grp in enumerate(TGROUPS)
                  for j, h in enumerate(grp)}
            ebt_sb = [cp.tile([128, N], f16, tag=f"ebt{g}", name=f"ebt{g}")
                      for g in range(3)]
            for g in range(3):
                nc.sync.dma_start(ebt_sb[g][:], expbt_d[g])

            # ---- persistent activation tiles ----
            q_sb = [[pp.tile([128, N], f16, tag=f"q{b}_{h}", name=f"q{b}_{h}")
                     for h in range(NH)] for b in range(BPC)]
            k_sb = [[pp.tile([128, N], f16, tag=f"k{b}_{h}", name=f"k{b}_{h}")
                     for h in range(NH)] for b in range(BPC)]
            vT_sb = [[pp.tile([128, 640], f16, tag=f"vT{b}_{m}", name=f"vT{b}_{m}")
                      for m in range(NMT + 1)] for b in range(BPC)]
            eTt_sb = [[pp.tile([128, N], f16, tag=f"eTt{b}_{g}", name=f"eTt{b}_{g}")
                       for g in range(3)] for b in range(BPC)]
            ounT = [pp.tile([65, NH, N], f16, tag=f"ounT{b}", name=f"ounT{b}")
                    for b in range(BPC)]
            onorm = [[pp.tile([128, N], f16, tag=f"onorm{b}_{p}", name=f"onorm{b}_{p}")
                      for p in range(4)] for b in range(BPC)]
            s_sh = pp.tile([12, NP], f16, tag="s", name="s")
            s32_sh = pp.tile([12, NP], f32, tag="s32", name="s32")
            r_sh = pp.tile([12, NP], f32, tag="r", name="r")
            r16_sh = pp.tile([12, NP], f16, tag="r16", name="r16")
            s2_sh = pp.tile([4, NP], f16, tag="s2", name="s2")
            s2_32_sh = pp.tile([4, NP], f32, tag="s232", name="s232")
            r2_sh = pp.tile([4, NP], f32, tag="r2", name="r2")
            r2_16_sh = pp.tile([4, NP], f16, tag="r216", name="r216")
            s_all = [s_sh, s_sh]
            s32 = [s32_sh, s32_sh]
            r_all = [r_sh, r_sh]
            r16 = [r16_sh, r16_sh]
            s2_all = [s2_sh, s2_sh]
            s2_32 = [s2_32_sh, s2_32_sh]
            r2_all = [r2_sh, r2_sh]
            r2_16 = [r2_16_sh, r2_16_sh]
            bc_all = pp.tile([48, NH * N], f16, tag="bc", name="bc")
            sg_dram = [dp.tile([12, NP], f16, tag=f"sg{b}", name=f"sg{b}")
                       for b in range(BPC)]
            r_dram = [dp.tile([12, NP], f16, tag=f"rd{b}", name=f"rd{b}")
                      for b in range(BPC)]
            sg2_dram = [dp.tile([4, NP], f16, tag=f"sg2{b}", name=f"sg2{b}")
                        for b in range(BPC)]
            r2_dram = [dp.tile([4, NP], f16, tag=f"rd2{b}", name=f"rd2{b}")
                       for b in range(BPC)]

            def init_tiles(b):
                # ones col @64 of each 80-col head block; onorm pad rows zeroed
                for m in range(NMT + 1):
                    nc.gpsimd.memset(
                        vT_sb[b][m].rearrange("p (h c) -> p h c", c=80)[:, :, 64:65],
                        1.0)
                for p in range(4):
                    nc.gpsimd.memset(onorm[b][p][32:64, :], 0.0)
                    nc.gpsimd.memset(onorm[b][p][96:128, :], 0.0)

            init_tiles(0)

            # ---- building blocks ----
            def qk_proj(b, p, wt, bt, src, dst, evac_eng):
                ps = psqk.tile([128, 1024], f32, tag="qk", name="psqk")
                for nch in range(2):
                    for t in range(3):
                        nc.tensor.matmul(
                            ps[:, 512 * nch:512 * nch + NP],
                            wt[t][:, 128 * p:128 * (p + 1)],
                            src[:, t, NP * nch:NP * (nch + 1)],
                            start=(t == 0),
                            stop=(t == 2),
                        )
                evac_eng.tensor_scalar(
                    dst[2 * p].rearrange("p (c n) -> p c n", c=2),
                    ps.rearrange("p (c n) -> p c n", n=512)[:, :, 0:NP],
                    bt[:, p:p + 1], None, ADD,
                )
                # duplicate rows to build per-head K=128 tiles
                nc.gpsimd.dma_start(dst[2 * p + 1][0:64, :], dst[2 * p][64:128, :])
                nc.gpsimd.dma_start(dst[2 * p + 1][64:128, :], dst[2 * p][64:128, :])
                nc.gpsimd.dma_start(dst[2 * p][64:128, :], dst[2 * p][0:64, :])

            def v_proj(b, mi):
                off = 128 * mi
                msz = 128 if mi < NMT else 16
                ps = psqk.tile([128, 1024], f32, tag="qk", name="psv")
                for t in range(3):
                    nc.tensor.matmul(
                        ps[0:msz, 0:384],
                        ha_sb[b][:, t, off:off + msz],
                        vwT_sb[t][:],
                        start=(t == 0),
                        stop=(t == 2),
                    )
                nc.vector.tensor_copy(
                    vT_sb[b][mi].rearrange("p (h c) -> p h c", c=80)[0:msz, :, 0:48],
                    ps[:, 0:384].rearrange("p (h c) -> p h c", c=48)[0:msz],
                )

            def v_tail_replicas(b):
                for r in (32, 64):
                    nc.gpsimd.dma_start(vT_sb[b][NMT][r:r + 16, :],
                                        vT_sb[b][NMT][0:16, :])

            def tailgrp(b, g):
                # QK + exp + bias-mult for the m-tail rows of a head group
                qkt = psqk.tile([128, 1024], f32, tag="qk", name="psqt")
                for j, h in enumerate(TGROUPS[g]):
                    for nch in range(2):
                        nc.tensor.matmul(
                            qkt[32 * j:32 * j + 16, 512 * nch:512 * nch + NP],
                            k_sb[b][h][:, 768:784],
                            q_sb[b][h][:, NP * nch:NP * (nch + 1)],
                            start=True, stop=True,
                        )
                nc.scalar.activation(
                    eTt_sb[b][g].rearrange("p (c n) -> p c n", c=2),
                    qkt.rearrange("p (c n) -> p c n", n=512)[:, :, 0:NP],
                    AF.Exp)
                nc.vector.tensor_tensor(
                    eTt_sb[b][g][:], eTt_sb[b][g][:], ebt_sb[g][:], MUL)

            def attend(h, b):
                g, j = TG[h]
                eb_sb = eb_tiles[h]
                av = psav.tile([65, 1024], f32, tag="av", name="avt")
                # m-tail first (opens the accumulation groups)
                for nch in range(2):
                    nc.tensor.matmul(
                        av[:, 512 * nch:512 * nch + NP],
                        vT_sb[b][NMT][32 * j:32 * j + 16, 80 * h:80 * h + 65],
                        eTt_sb[b][g][32 * j:32 * j + 16, NP * nch:NP * (nch + 1)],
                        start=True, stop=False,
                    )
                for mi in range(NMT):
                    off = 128 * mi
                    eT = etp.tile([128, N], f16, tag="eT", bufs=3, name="eTt")
                    qk = psqk.tile([128, 1024], f32, tag="qk", name="qkt")
                    for nch in range(2):
                        nc.tensor.matmul(
                            qk[:, 512 * nch:512 * nch + NP],
                            k_sb[b][h][:, off:off + 128],
                            q_sb[b][h][:, NP * nch:NP * (nch + 1)],
                            start=True, stop=True,
                        )
                    nc.scalar.activation(
                        eT.rearrange("p (c n) -> p c n", c=2),
                        qk.rearrange("p (c n) -> p c n", n=512)[:, :, 0:NP],
                        AF.Exp)
                    nc.vector.tensor_tensor(eT[:], eT[:], eb_sb[:, mi, :], MUL)
                    for nch in range(2):
                        nc.tensor.matmul(
                            av[:, 512 * nch:512 * nch + NP],
                            vT_sb[b][mi][:, 80 * h:80 * h + 65],
                            eT[:, NP * nch:NP * (nch + 1)],
                            start=False, stop=(mi == NMT - 1),
                        )
                nc.vector.tensor_copy(
                    ounT[b][:, h, :].rearrange("p (c n) -> p c n", c=2),
                    av.rearrange("p (c n) -> p c n", n=512)[:, :, 0:NP],
                )

            def tail1(b):
                # heads 0-5: softmax denominators ready after (5, b)
                nc.gpsimd.dma_start(
                    sg_dram[b].rearrange("p n -> () (p n)").rearrange(
                        "() (h n) -> () h n", n=N),
                    ounT[b][64:65, 0:6, :],
                )
                nc.gpsimd.dma_start(s_all[b][:], sg_dram[b][:])
                nc.vector.tensor_copy(s32[b][:], s_all[b][:])
                nc.vector.reciprocal_approx_fast(r_all[b][:], s32[b][:])
                nc.vector.tensor_copy(r16[b][:], r_all[b][:])
                nc.gpsimd.dma_start(r_dram[b][:], r16[b][:])
                nc.gpsimd.dma_start(
                    bc_all[:, 0:6 * N],
                    r_dram[b].tensor.ap().rearrange(
                        "p n -> () (p n)").to_broadcast((48, 6 * N)),
                )
                for h in range(6):
                    prr, hpp = divmod(h, 2)
                    nc.vector.tensor_tensor(
                        onorm[b][prr][64 * hpp:64 * hpp + 48, :],
                        ounT[b][0:48, h, :],
                        bc_all[:, N * h:N * (h + 1)],
                        MUL,
                    )

            def tail2(b):
                # heads 6-7 after (7, b)
                nc.gpsimd.dma_start(
                    sg2_dram[b].rearrange("p n -> () (p n)").rearrange(
                        "() (h n) -> () h n", n=N),
                    ounT[b][64:65, 6:8, :],
                )
                nc.gpsimd.dma_start(s2_all[b][:], sg2_dram[b][:])
                nc.vector.tensor_copy(s2_32[b][:], s2_all[b][:])
                nc.vector.reciprocal_approx_fast(r2_all[b][:], s2_32[b][:])
                nc.vector.tensor_copy(r2_16[b][:], r2_all[b][:])
                nc.gpsimd.dma_start(r2_dram[b][:], r2_16[b][:])
                nc.gpsimd.dma_start(
                    bc_all[:, 6 * N:8 * N],
                    r2_dram[b].tensor.ap().rearrange(
                        "p n -> () (p n)").to_broadcast((48, 2 * N)),
                )
                for h in (6, 7):
                    prr, hpp = divmod(h, 2)
                    nc.vector.tensor_tensor(
                        onorm[b][prr][64 * hpp:64 * hpp + 48, :],
                        ounT[b][0:48, h, :],
                        bc_all[:, N * h:N * (h + 1)],
                        MUL,
                    )

            def proj(b):
                for o in range(3):
                    ps = psav.tile([128, 1024], f32, tag="av", name="psy")
                    for nch in range(2):
                        for p in range(4):
                            nc.tensor.matmul(
                                ps[:, 512 * nch:512 * nch + NP],
                                pwT_sb[p][:, 128 * o:128 * (o + 1)],
                                onorm[b][p][:, NP * nch:NP * (nch + 1)],
                                start=(p == 0), stop=(p == 3),
                            )
                    y_sb = ypool.tile([128, N], f32, tag="y", name="ysb")
                    if b == 0:
                        nc.vector.tensor_scalar(
                            y_sb.rearrange("p (c n) -> p c n", c=2),
                            ps.rearrange("p (c n) -> p c n", n=512)[:, :, 0:NP],
                            pb_sb[:, o:o + 1], None, ADD)
                    else:
                        nc.scalar.activation(
                            y_sb.rearrange("p (c n) -> p c n", c=2),
                            ps.rearrange("p (c n) -> p c n", n=512)[:, :, 0:NP],
                            AF.Identity, bias=pb_sb[:, o:o + 1])
                    nc.sync.dma_start(
                        out_d[b, 128 * o:128 * (o + 1), 0:NP], y_sb[:, 0:NP])
                    nc.gpsimd.dma_start(
                        out_d[b, 128 * o:128 * (o + 1), NP:N], y_sb[:, NP:N])

            # ---- emission schedule ----
            qk_proj(0, 0, qwT_sb, qb_sb, ll_sb[0], q_sb[0], nc.vector)
            qk_proj(0, 0, kwT_sb, kb_sb, ha_sb[0], k_sb[0], nc.vector)
            qk_proj(0, 1, qwT_sb, qb_sb, ll_sb[0], q_sb[0], nc.vector)
            qk_proj(0, 1, kwT_sb, kb_sb, ha_sb[0], k_sb[0], nc.vector)
            for mi in range(NMT + 1):
                v_proj(0, mi)
            v_tail_replicas(0)
            load_act(1)
            tailgrp(0, 0)
            init_tiles(1)

            attend(0, 0)
            qk_proj(0, 2, qwT_sb, qb_sb, ll_sb[0], q_sb[0], nc.vector)
            qk_proj(0, 2, kwT_sb, kb_sb, ha_sb[0], k_sb[0], nc.vector)
            qk_proj(0, 3, qwT_sb, qb_sb, ll_sb[0], q_sb[0], nc.vector)
            qk_proj(0, 3, kwT_sb, kb_sb, ha_sb[0], k_sb[0], nc.vector)
            load_eb(2)
            attend(1, 0)
            tailgrp(0, 1)
            tailgrp(0, 2)
            qk_proj(1, 0, qwT_sb, qb_sb, ll_sb[1], q_sb[1], nc.vector)
            qk_proj(1, 0, kwT_sb, kb_sb, ha_sb[1], k_sb[1], nc.vector)
            qk_proj(1, 1, qwT_sb, qb_sb, ll_sb[1], q_sb[1], nc.vector)
            qk_proj(1, 1, kwT_sb, kb_sb, ha_sb[1], k_sb[1], nc.vector)
            for mi in range(NMT + 1):
                v_proj(1, mi)
            v_tail_replicas(1)
            tailgrp(1, 0)
            attend(0, 1)
            load_eb(3)
            attend(2, 0)
            qk_proj(1, 2, qwT_sb, qb_sb, ll_sb[1], q_sb[1], nc.vector)
            qk_proj(1, 2, kwT_sb, kb_sb, ha_sb[1], k_sb[1], nc.vector)
            qk_proj(1, 3, qwT_sb, qb_sb, ll_sb[1], q_sb[1], nc.vector)
            qk_proj(1, 3, kwT_sb, kb_sb, ha_sb[1], k_sb[1], nc.vector)
            attend(1, 1)
            tailgrp(1, 1)
            tailgrp(1, 2)
            load_eb(4)
            attend(3, 0)
            attend(2, 1)
            load_eb(5)
            attend(4, 0)
            attend(3, 1)
            load_eb(6)
            attend(5, 0)
            tail1(0)
            attend(4, 1)
            load_eb(7)
            attend(6, 0)
            attend(5, 1)
            tail1(1)
            attend(7, 0)
            tail2(0)
            attend(6, 1)
            attend(7, 1)
            proj(0)
            tail2(1)
            proj(1)

    nc.finalize()
    return nc


def _prep_consts(q_w, q_b, kv_w, kv_b, proj_w, proj_b, attn_biases, bias_idxs):
    f16 = np.float16
    qw = (q_w * SCALE).astype(np.float32)
    qb = (q_b * SCALE).astype(np.float32)
    kw = kv_w[:C] * 0.5
    kb = kv_b[:C] * 0.5
    vw = kv_w[C:]
    vb = kv_b[C:]

    def pad64(w2, b1):  # [384(o), 384(c)] -> [512, 384] / [512]
        wp = np.zeros((512, C), np.float32)
        bp = np.zeros((512,), np.float32)
        for h in range(NH):
            wp[64 * h:64 * h + HD] = w2[HD * h:HD * (h + 1)]
            bp[64 * h:64 * h + HD] = b1[HD * h:HD * (h + 1)]
        return wp, bp

    qwp, qbp = pad64(qw, qb)
    kwp, kbp = pad64(kw, kb)
    qwT = np.ascontiguousarray(qwp.T.reshape(3, 128, 512)).astype(f16)
    kwT = np.ascontiguousarray(kwp.T.reshape(3, 128, 512)).astype(f16)
    vwT = np.ascontiguousarray(vw.T.reshape(3, 128, C)).astype(f16)

    # proj weights in onorm pair-tile layout: pair p row j -> channel
    pwT = np.zeros((4, 128, C), np.float32)
    for p in range(4):
        pwT[p, 0:HD] = proj_w[:, 96 * p:96 * p + HD].T
        pwT[p, 64:64 + HD] = proj_w[:, 96 * p + HD:96 * p + 96].T
    pwT = pwT.astype(f16)

    qb_h = np.ascontiguousarray(qbp.reshape(4, 128).T).astype(np.float32)
    kb_h = np.ascontiguousarray(kbp.reshape(4, 128).T).astype(np.float32)
    # v-bias folded into the projection bias: y = Wp@(out/s) + (Wp@vb + pb)
    pb2 = (proj_b + proj_w @ vb).astype(np.float32)
    pb_h = np.ascontiguousarray(pb2.reshape(3, 128).T).astype(np.float32)

    eb = np.exp(attn_biases[:, bias_idxs]).astype(np.float32)  # [NH, N, N]
    expb = np.ascontiguousarray(
        eb[:, :NMT * 128, :].reshape(NH, NMT, 128, N)).astype(f16)
    expbt = np.ones((3, 128, N), np.float32)
    for g, grp in enumerate([(0, 1, 2), (3, 4, 5), (6, 7)]):
        for j, h in enumerate(grp):
            expbt[g, 32 * j:32 * j + 16] = eb[h, 768:784, :]
    expbt = expbt.astype(f16)

    return dict(qwT=qwT, kwT=kwT, vwT=vwT, pwT=pwT, qb=qb_h, kb=kb_h,
                pb=pb_h, expb=expb, expbt=expbt)


def kernel(ll, high_attn, q_w, q_b, kv_w, kv_b, proj_w, proj_b,
           attn_biases, bias_idxs):
    from concourse.bass_utils import run_bass_kernel_spmd

    global LAST_RESULTS
    ll = np.asarray(ll)
    high_attn = np.asarray(high_attn)

    if "nc" not in _CACHE:
        _CACHE["nc"] = _build_nc()
    nc = _CACHE["nc"]

    consts = _prep_consts(
        np.asarray(q_w), np.asarray(q_b), np.asarray(kv_w), np.asarray(kv_b),
        np.asarray(proj_w), np.asarray(proj_b), np.asarray(attn_biases),
        np.asarray(bias_idxs),
    )

    ll16 = ll.reshape(B, C, N).astype(np.float16)
    ha16 = high_attn.reshape(B, C, N).astype(np.float16)

    in_maps = []
    for i in range(NCORES):
        m = {"ll": ll16[BPC * i:BPC * (i + 1)], "ha": ha16[BPC * i:BPC * (i + 1)]}
        m.update(consts)
        in_maps.append(m)

    res = run_bass_kernel_spmd(nc, in_maps, core_ids=list(range(NCORES)),
                               trace=TRACE)
    LAST_RESULTS = {"exec_time_ns": res.exec_time_ns,
                    "scope_times": res.per_core_scope_times}

    out = np.empty((B, C, N), np.float32)
    for i in range(NCORES):
        out[BPC * i:BPC * (i + 1)] = res.results[i]["out"]
    return out.reshape(B, C, RES, RES)
